# revision 22
# baseline (speedup 1.0000x reference)
"""Trainium2 Bass kernel for nn_NeuralODECNN (RK4 neural-ODE CNN forward).

Self-contained: hardcodes all shapes. Data-parallel over batch across 8
NeuronCores (16 images per core); all params replicated.

Per-core on-chip layouts (B_local=16 images, j = g*4 + s, g=group, s=slot):
  folded  [128 = g*32 + c, 4096 = s*1024 + y*32 + x]   (z / k / x tensors, c<32)
  h1pad   [128, 16, 34, 34]  zero-padded per image for the 3x3 conv taps
  h2b     per-slot [128, 4096 = g*1024 + y*32 + x]     (bufs=2)

v6 design (from v5 1553us baseline; PE 81%/ScalarE 79% busy, 302us HAM
throttle):
 - tanh moved OFF ScalarE onto a custom DVE op (TANH7_ANT: odd deg-7
   Horner polynomial, 8 ALU stages, 1 uop, coefficients fit on [-3,3]
   with the input pre-clamped by the psum drain). This deletes all four
   ACT_TABLE_LOADs per eval (~10.6us) plus the tanh passes; ScalarE
   stays pinned on natural_log_exp_and_others for the whole kernel.
 - the 16 RK4 sub-evals run as ONE continuous image stream: conv1 of
   eval e+1 overlaps conv2/conv3 of eval e (conv1 leads conv2 by LEAD=8
   image-steps). No per-eval barrier -> PE never idles long enough for
   the HAM clock gate to re-throttle.
 - slot-finalize (ln2, conv3, drains, tanh-poly, RK4 combine) is emitted
   one image-step late so ScalarE's ln2 is already done when the PE
   reaches the conv3 matmuls.
 - conv3 drain applies bias+clamp via dual-scalar-op tensor_scalar ops.
 - readout: 4-way col-tiled (tile_position) accumulating matmuls, strip
   sum via a one-hot matmul; per-slot refold DMAs issued right after the
   slot's last combine so they overlap the remaining eval-15 compute.

softplus stays Exp+Ln on ScalarE (natural_log_exp set): there is no
softplus HW table on this arch (compile-checked), and no 8-stage DVE
polynomial can capture its exponential tails.
"""

import os
from contextlib import ExitStack

import bass_rust
import ml_dtypes
import numpy as np

import concourse.bacc as bacc
import concourse.mybir as mybir
import concourse.tile as tile
from concourse.bass_utils import run_bass_kernel_spmd
from concourse.hw_specs import get_activation_tables

F32 = mybir.dt.float32
AF = mybir.ActivationFunctionType
ALU = mybir.AluOpType

NCORES = 8
BL = 16          # images per core
STEPS = 4        # RK4 steps (= pieces, STEPS_PER_PIECE=1, dt=1)
DT_NAME = os.environ.get("ODE_DT", "bf16")   # bf16 | fp32r | fp32
LEAD = 8         # conv1 image-steps ahead of conv2 in the global stream

# tanh ~= x*(A1 + u*(A2 + u*(A3 + u*A4))), u=x^2, input clamped to [-R,R]
TANH_A = (9.439652037e-01, -1.987238189e-01, 2.518041247e-02, -1.191157362e-03)
TANH_R = 3.0

# eval schedule: step i evals use t = i + {0,.5,.5,1}, piece = i (k1..k3) or
# min(i+1,3) (k4, since floor(i+1) indexes the next piece)
_EVAL_TP = [(i + dt, i if k < 3 else min(i + 1, 3))
            for i in range(4) for k, dt in enumerate((0.0, 0.5, 0.5, 1.0))]


def _register_tanh_op():
    """Register TANH7_ANT in concourse.dve_ops at import time (documented
    custom-DVE extension point; per-NEFF table is generated automatically
    from Module.ant_custom_dve_ops)."""
    import concourse.dve_ops as dve_ops
    for op in dve_ops.OPS:
        if op.name == "TANH7_ANT":
            return op
    from concourse.dve_spec import (Spec, Src0, C0, C1, C2, C3, sq, lower,
                                    _spill_c3_to_src1)
    from concourse.dve_uop import DveOpSpec

    # a4 rides C3, spilled to a Latch(Src1) read once at element 0 (the
    # production-proven [P,1] in1 mechanism; a *streamed* [P,1] Src1
    # broadcast hard-faults on HW).
    u = sq(Src0)
    body = _spill_c3_to_src1(((((C3 * u) + C2) * u + C1) * u + C0) * Src0)

    def ref(in0, in1, s0, s1, imm2):
        x = in0.astype(np.float32)
        uu = x * x
        return ((((in1 * uu) + imm2) * uu + s1) * uu + s0) * x

    spec = Spec(body=body, reference=ref)
    row = dve_ops._CUSTOM_DVE_ROW_BASE + len(dve_ops.OPS)
    assert row < 0x20
    shas = {}
    for ver in ("v3", "v4"):
        s = DveOpSpec(name="TANH7_ANT", opcode=row, uops=lower(spec, ver=ver),
                      rd1_en=True)
        shas[ver] = s.sha(ver)
    op = dve_ops.DveOp("TANH7_ANT", spec, subdim=False, uops_sha=shas)
    dve_ops.OPS.append(op)
    dve_ops._SUB_OPCODE_FOR_NAME[op.name] = row
    dve_ops.CUSTOM_DVE_SPECS[op.name] = spec
    return op


TANH7 = _register_tanh_op()


def _mm_dtype():
    return {"bf16": mybir.dt.bfloat16, "fp32r": F32, "fp32": F32}[DT_NAME]


def build_nc(debug=False):
    DT = _mm_dtype()
    if DT_NAME == "fp32r":
        cast = lambda ap: ap.bitcast(mybir.dt.float32r)  # noqa: E731
    else:
        cast = lambda ap: ap  # noqa: E731

    if os.environ.get("ODE_SIM"):
        nc = bacc.Bacc("TRN2", target_bir_lowering=False, debug=True)
    else:
        nc = bacc.Bacc("TRN2")

    _set_names = list(get_activation_tables(nc.m.arch).keys())
    SET_NLE = _set_names.index("natural_log_exp_and_others")   # exp + ln

    # All ScalarE instructions are chained with nosync deps in emission
    # order (engine is FIFO anyway); the single explicit table pin at the
    # top stops the auto table-load pass from inserting further loads.
    _pin_n = [0]
    _last_scalar = [None]

    def _chain_scalar(raw):
        if _last_scalar[0] is not None:
            raw.add_nosync_dependencies_from(
                bass_rust.InstructionNameOrderedSet([_last_scalar[0]]))
        _last_scalar[0] = raw.name

    def sact(*args, **kwargs):
        r = nc.scalar.activation(*args, **kwargs)
        _chain_scalar(r.ins)
        return r

    def pin_table(set_id):
        _pin_n[0] += 1
        ins = bass_rust.InstLoadActFuncSet(
            act_func_set_id=set_id, name=f"pin_act_{_pin_n[0]}")
        nc.scalar.add_instruction(ins)
        _chain_scalar(ins)

    xf_d = nc.dram_tensor("xf", [128, 4096], DT, kind="ExternalInput")
    # conv1/aug weights are zero-padded to full K=128 (rows outside the
    # group's 32-partition strip are zero).
    w1_d = nc.dram_tensor("w1s", [4, 4, 128, 128], DT, kind="ExternalInput")
    w2_d = nc.dram_tensor("w2s", [4, 9, 128, 128], DT, kind="ExternalInput")
    w3_d = nc.dram_tensor("w3s", [4, 128, 32], DT, kind="ExternalInput")
    aw_d = nc.dram_tensor("augw", [4, 128, 32], DT, kind="ExternalInput")
    b1_d = nc.dram_tensor("b1e", [128, 16], F32, kind="ExternalInput")
    b2_d = nc.dram_tensor("b2s", [128, 4], F32, kind="ExternalInput")
    b3_d = nc.dram_tensor("b3s", [128, 4], F32, kind="ExternalInput")
    ab_d = nc.dram_tensor("augb", [128, 1], F32, kind="ExternalInput")
    ro_d = nc.dram_tensor("row", [128, 2560], DT, kind="ExternalInput")
    sel_d = nc.dram_tensor("sel", [128, 10], F32, kind="ExternalInput")
    oh_d = nc.dram_tensor("oneh", [16, 10], F32, kind="ExternalInput")
    rb_d = nc.dram_tensor("rob", [16, 10], F32, kind="ExternalInput")
    out_d = nc.dram_tensor("outv", [2, 1], F32, kind="ExternalOutput")
    if debug:
        zf_d = nc.dram_tensor("zf", [128, 4096], F32, kind="ExternalOutput")
        lg_d = nc.dram_tensor("lg", [16, 10], F32, kind="ExternalOutput")

    n_evals = int(os.environ.get("ODE_NEVALS", "16"))

    with tile.TileContext(nc) as tc, ExitStack() as ctx:
        sing = ctx.enter_context(tc.tile_pool(name="sing", bufs=1))
        z = sing.tile([128, 4096], F32)
        zin = sing.tile([128, 4096], DT)
        acc = sing.tile([128, 4096], F32)
        t0b = sing.tile([128, 4096], F32)
        w1b = sing.tile([128, 4, 4, 128], DT)
        w2b = sing.tile([128, 4, 9, 128], DT)
        w3b = sing.tile([128, 4, 32], DT)
        awb = sing.tile([128, 4, 32], DT)
        b1b = sing.tile([128, 16], F32)
        b2b = sing.tile([128, 4], F32)
        b3b = sing.tile([128, 4], F32)
        abb = sing.tile([128, 1], F32)
        a4t = sing.tile([128, 1], F32)
        zR = sing.tile([128, 4096], DT)
        rob_w = sing.tile([128, 2560], DT)
        selb = sing.tile([128, 10], F32)
        nc.vector.memset(a4t[:], TANH_A[3])

        with (
            tc.tile_pool(name="mid", bufs=1) as mid,
            tc.tile_pool(name="h2p", bufs=2) as h2p,
            tc.tile_pool(name="e3p", bufs=2) as e3p,
            tc.tile_pool(name="stp", bufs=1) as stp,
            tc.tile_pool(name="s4p", bufs=2) as s4p,
            tc.tile_pool(name="p1", bufs=1, space="PSUM") as p1p,
            tc.tile_pool(name="p2", bufs=2, space="PSUM") as p2p,
            tc.tile_pool(name="p3", bufs=2, space="PSUM") as p3p,
        ):
            h1pad = mid.tile([128, 16, 34, 34], DT)
            xfb = mid.tile([128, 4096], DT)

            # input DMAs ordered by first use: augment inputs first, the
            # big conv2 weights (needed only ~15us in) late, readout
            # weights at the very end (emitted in the readout block).
            nc.sync.dma_start(xfb[:], xf_d[:])
            nc.sync.dma_start(awb[:], aw_d.rearrange("g i m -> i g m"))
            nc.sync.dma_start(abb[:], ab_d[:])
            nc.sync.dma_start(b1b[:], b1_d[:])
            nc.sync.dma_start(w1b[:], w1_d.rearrange("p g i m -> i p g m"))
            nc.sync.dma_start(b2b[:], b2_d[:])
            nc.sync.dma_start(b3b[:], b3_d[:])
            nc.sync.dma_start(w3b[:], w3_d.rearrange("p i m -> i p m"))
            nc.sync.dma_start(w2b[:], w2_d.rearrange("p t i m -> i p t m"))
            nc.sync.dma_start(selb[:], sel_d[:])
            nc.vector.memset(h1pad[:], 0.0)
            pin_table(SET_NLE)

            # ---- augment: z0 = aug_W @ x + aug_b (col-tiled, zero-pad K) ----
            for s in range(4):
                ps = p1p.tile([128, 1024], F32, tag="ps1")
                for g in range(4):
                    for h in range(2):
                        n0 = s * 1024 + h * 512
                        nc.tensor.matmul(
                            ps[32 * g:32 * g + 32, h * 512:(h + 1) * 512],
                            cast(awb[:, g, :]),
                            cast(xfb[:, n0:n0 + 512]),
                            start=True, stop=True, tile_position=(0, 32 * g))
                sl = slice(s * 1024, (s + 1) * 1024)
                sact(z[:, sl], ps[:], AF.Identity, bias=abb[:, 0:1])
                nc.vector.tensor_copy(zin[:, sl], z[:, sl])

            # ---- the 16 RK4 sub-evals as one continuous image stream ----
            imgs = [g * 4 + s for s in range(4) for g in range(4)]  # s-major

            def conv1(j, piece, eidx):
                g, s = j // 4, j % 4
                ps1 = p1p.tile([128, 1024], F32, tag="ps1")
                for h in range(2):
                    n0 = s * 1024 + h * 512
                    nc.tensor.matmul(
                        ps1[:, h * 512:(h + 1) * 512],
                        cast(w1b[:, piece, g, :]),
                        cast(zin[:, n0:n0 + 512]),
                        start=True, stop=True)
                st = stp.tile([128, 1024], F32, tag="st")
                sact(st[:], ps1[:], AF.Exp, bias=b1b[:, eidx:eidx + 1])
                sact(h1pad[:, j, 1:33, 1:33],
                     st.rearrange("p (a b) -> p a b", b=32),
                     AF.Ln, bias=1.0)

            def conv2(j, piece, st4b):
                g, s = j // 4, j % 4
                ps2 = p2p.tile([128, 1024], F32, tag="ps2")
                for tap in range(9):
                    dy, dx = tap // 3, tap % 3
                    for h in range(2):
                        y0 = h * 16 + dy
                        nc.tensor.matmul(
                            ps2[:, h * 512:(h + 1) * 512],
                            cast(w2b[:, piece, tap, :]),
                            cast(h1pad[:, j, y0:y0 + 16, dx:dx + 32]),
                            start=(tap == 0), stop=(tap == 8))
                sact(st4b[:, g * 1024:(g + 1) * 1024], ps2[:], AF.Exp,
                     bias=b2b[:, piece:piece + 1])

            def finalize(s, eidx, st4b):
                piece = _EVAL_TP[eidx][1]
                e, last = eidx % 4, (eidx // 4 == STEPS - 1)
                # softplus tail of conv2 for the whole slot (4 images)
                h2b = h2p.tile([128, 4096], DT, tag="h2")
                sact(h2b[:], st4b[:], AF.Ln, bias=1.0)
                # conv3 (128ch -> 32ch, col-tiled 4-way) + biased clamped drain
                e3b = e3p.tile([128, 1024], F32, tag="e3")
                for half in range(2):
                    ps3 = p3p.tile([128, 512], F32, tag="ps3")
                    for g in range(4):
                        n0 = g * 1024 + half * 512
                        nc.tensor.matmul(
                            ps3[32 * g:32 * g + 32, :],
                            cast(w3b[:, piece, :]), cast(h2b[:, n0:n0 + 512]),
                            start=True, stop=True, tile_position=(0, 32 * g))
                    hs = slice(half * 512, (half + 1) * 512)
                    nc.vector.tensor_scalar(
                        e3b[:, hs], ps3[:], b3b[:, piece:piece + 1], TANH_R,
                        ALU.add, ALU.min)
                    nc.vector.tensor_scalar(
                        e3b[:, hs], e3b[:, hs], -TANH_R, None, ALU.max)
                # k = tanh(e3b) via the custom DVE polynomial
                sl = slice(s * 1024, (s + 1) * 1024)
                dst = acc if e == 0 else t0b
                nc.vector._custom_dve(
                    TANH7, out=dst[:, sl], in0=e3b[:], in1=a4t[:, 0:1],
                    s0=TANH_A[0], s1=TANH_A[1], imm2=TANH_A[2])
                # RK4 combine for slot s (dt = 1)
                stt = nc.vector.scalar_tensor_tensor
                tt = nc.vector.tensor_tensor
                k = t0b[:, sl]
                if e == 0:
                    stt(zin[:, sl], acc[:, sl], 0.5, z[:, sl], ALU.mult, ALU.add)
                elif e == 1:
                    stt(zin[:, sl], k, 0.5, z[:, sl], ALU.mult, ALU.add)
                    stt(acc[:, sl], k, 2.0, acc[:, sl], ALU.mult, ALU.add)
                elif e == 2:
                    tt(zin[:, sl], z[:, sl], k, ALU.add)
                    stt(acc[:, sl], k, 2.0, acc[:, sl], ALU.mult, ALU.add)
                else:
                    tt(acc[:, sl], acc[:, sl], k, ALU.add)
                    stt(z[:, sl], acc[:, sl], 1.0 / 6.0, z[:, sl],
                        ALU.mult, ALU.add)
                    # on the last eval this copy is the bf16 staging for
                    # the readout refold rather than the next eval's input
                    nc.vector.tensor_copy(zin[:, sl], z[:, sl])
                if eidx == n_evals - 1:
                    # refold this slot for the bf16 readout:
                    # zR[t*32+c, (g*4+s)*256+q] = zin[g*32+c, s*1024+t*256+q]
                    # spread across engine DMA queues so the last slot's 16
                    # transfers don't serialize on one ring
                    engs = [nc.sync, nc.gpsimd, nc.scalar]
                    for di, (g, t) in enumerate((g, t) for g in range(4)
                                                for t in range(4)):
                        eng = engs[di % 3]
                        r = eng.dma_start(
                            zR.rearrange("p (gg ss q) -> p gg ss q",
                                         ss=4, q=256)
                              [32 * t:32 * t + 32, g, s, :],
                            zin.rearrange("p (ss tt q) -> p ss tt q",
                                          tt=4, q=256)
                              [32 * g:32 * g + 32, s, t, :])
                        if eng is nc.scalar:
                            _chain_scalar(r.ins)

            # pending finalize is emitted one image-step later so ScalarE's
            # ln2 completes before the PE reaches the conv3 matmuls.
            # conv2 trails conv1 by 4 image-steps during eval 0 (no
            # cross-eval dependency yet) and by LEAD=8 from eval 1 on.
            pending = None
            st4b_cur = [None]
            next_q = 0
            n_steps = 16 * n_evals
            p = 0
            while next_q < n_steps or p < n_steps:
                if p < n_steps:
                    e1, i1 = divmod(p, 16)
                    conv1(imgs[i1], _EVAL_TP[e1][1], e1)
                lead = 4 if next_q < 8 else LEAD
                if next_q < n_steps and p - next_q >= lead:
                    q = next_q
                    next_q += 1
                    e2, i2 = divmod(q, 16)
                    if i2 % 4 == 0:
                        st4b_cur[0] = s4p.tile([128, 4096], F32, tag="st4",
                                               name=f"st4_{q}")
                    conv2(imgs[i2], _EVAL_TP[e2][1], st4b_cur[0])
                    if pending is not None:
                        finalize(*pending)
                        pending = None
                    if i2 % 4 == 3:
                        pending = (i2 // 4, e2, st4b_cur[0])
                p += 1
            if pending is not None:
                finalize(*pending)

        # ---- readout: logits, loss, accuracy ----
        with (
            tc.tile_pool(name="ro", bufs=1) as rop,
            tc.tile_pool(name="pro", bufs=1, space="PSUM") as prop,
        ):
            ohb = rop.tile([16, 10], F32)
            rbb = rop.tile([16, 10], F32)
            nc.sync.dma_start(rob_w[:], ro_d[:])
            nc.sync.dma_start(ohb[:], oh_d[:])
            nc.sync.dma_start(rbb[:], rb_d[:])
            if debug:
                nc.sync.dma_start(zf_d[:], z[:])

            ro_mode = os.environ.get("ODE_RO", "full")
            lt = rop.tile([32, 32], F32)
            ltT = rop.tile([32, 32], F32)
            ltS = rop.tile([128, 16], F32)
            nc.vector.memset(lt[:], 0.0)
            nc.vector.memset(ltS[:], 0.0)
            if ro_mode in ("full", "nostat"):
                # 4-way col-tiled accumulation over the 256 q-chunks; one
                # psum tile (= one bank) per col-tile strip so each strip is
                # its own accumulation group.
                lg_tiles = [prop.tile([128, 16], F32, tag=f"lgp{c}",
                                      name=f"lg_ps{c}") for c in range(4)]
                zRq = zR.rearrange("p (j q) -> p j q", q=256)
                for r in range(64):
                    for c in range(4):
                        qq = r * 4 + c
                        nc.tensor.matmul(
                            lg_tiles[c][32 * c:32 * c + 10, :],
                            cast(rob_w[:, 10 * qq:10 * qq + 10]),
                            cast(zRq[:, :, qq]),
                            start=(r == 0), stop=(r == 63),
                            tile_position=(0, 32 * c))
                for c in range(4):
                    sact(ltS[32 * c:32 * c + 10, :],
                         lg_tiles[c][32 * c:32 * c + 10, :],
                         AF.Identity, bias=0.0)
                # sum the 4 strips: lg2 = sel.T @ ltS  ([10, 16])
                lg2_ps = prop.tile([10, 16], F32)
                nc.tensor.matmul(lg2_ps[:, :], selb[:], ltS[:],
                                 start=True, stop=True)
                sact(lt[0:10, 0:16], lg2_ps[:, :], AF.Identity, bias=0.0)
            nc.vector.transpose(ltT[:], lt[:])

            lgt = rop.tile([16, 10], F32)
            nc.vector.tensor_tensor(lgt[:], ltT[0:16, 0:10], rbb[:], ALU.add)
            if debug:
                nc.sync.dma_start(lg_d[:], lgt[:])

            if ro_mode in ("nostat", "nomm", "none"):
                sm0 = rop.tile([2, 1], F32)
                nc.vector.memset(sm0[:], 0.0)
                nc.sync.dma_start(out_d[:], sm0[:])
            else:
                mx = rop.tile([16, 1], F32)
                nc.vector.tensor_reduce(mx[:], lgt[:], mybir.AxisListType.X,
                                        ALU.max)
                sx = rop.tile([16, 10], F32)
                nc.vector.tensor_scalar(sx[:], lgt[:], mx[:], None,
                                        ALU.subtract)
                ex = rop.tile([16, 10], F32)
                sact(ex[:], sx[:], AF.Exp)
                se = rop.tile([16, 1], F32)
                nc.vector.tensor_reduce(se[:], ex[:], mybir.AxisListType.X,
                                        ALU.add)
                lse = rop.tile([16, 1], F32)
                sact(lse[:], se[:], AF.Ln)

                prod = rop.tile([16, 10], F32)
                tcorr = rop.tile([16, 1], F32)
                nc.vector.tensor_tensor(prod[:], lgt[:], ohb[:], ALU.mult)
                nc.vector.tensor_reduce(tcorr[:], prod[:], mybir.AxisListType.X,
                                        ALU.add)

                lossv = rop.tile([16, 1], F32)
                accv = rop.tile([16, 1], F32)
                nc.vector.tensor_tensor(lossv[:], lse[:], mx[:], ALU.add)
                nc.vector.tensor_tensor(lossv[:], lossv[:], tcorr[:],
                                        ALU.subtract)
                nc.vector.tensor_tensor(accv[:], mx[:], tcorr[:], ALU.is_equal)

                lv2 = rop.tile([128, 2], F32)
                nc.vector.memset(lv2[:], 0.0)
                nc.vector.tensor_copy(lv2[0:16, 0:1], lossv[:])
                nc.vector.tensor_copy(lv2[0:16, 1:2], accv[:])
                ones = rop.tile([128, 1], F32)
                nc.vector.memset(ones[:], 1.0)
                sm_ps = prop.tile([2, 1], F32)
                nc.tensor.matmul(sm_ps[:, :], lv2[:], ones[:],
                                 start=True, stop=True)
                sm = rop.tile([2, 1], F32)
                sact(sm[:], sm_ps[:, :], AF.Identity, bias=0.0)
                nc.sync.dma_start(out_d[:], sm[:])

    nc.compile()
    return nc


# ---------------- host-side input prep ----------------

def prep_in_maps(inputs):
    DT_np = {"bf16": ml_dtypes.bfloat16, "fp32r": np.float32,
             "fp32": np.float32}[DT_NAME]
    f = np.float32
    x = np.asarray(inputs["x"], f)          # [128, 3, 32, 32]
    y = np.asarray(inputs["y"]).astype(np.int64)  # [128]
    aug_W = np.asarray(inputs["aug_W"], f)  # [32, 3]
    aug_b = np.asarray(inputs["aug_b"], f)  # [32]
    W1 = np.asarray(inputs["W1"], f)        # [4, 128, 33]
    b1 = np.asarray(inputs["b1"], f)        # [4, 128]
    W2 = np.asarray(inputs["W2"], f)        # [4, 128, 128, 3, 3]
    b2 = np.asarray(inputs["b2"], f)        # [4, 128]
    W3 = np.asarray(inputs["W3"], f)        # [4, 32, 128]
    b3 = np.asarray(inputs["b3"], f)        # [4, 32]
    ro_W = np.asarray(inputs["ro_W"], f)    # [10, 32768]
    ro_b = np.asarray(inputs["ro_b"], f)    # [10]

    # xf: [core, 128 = g*32+c (c<3), 4096 = s*1024 + pos]
    xr = x.reshape(NCORES, 4, 4, 3, 1024)          # [core, g, s, c, pos]
    xf = np.zeros((NCORES, 4, 32, 4, 1024), f)     # [core, g, c, s, pos]
    xf[:, :, :3] = xr.transpose(0, 1, 3, 2, 4)
    xf = np.ascontiguousarray(xf.reshape(NCORES, 128, 4096)).astype(DT_np)

    # w1s[p, g, 32g+c, m] = W1[p, m, 1+c]; zero outside group g's strip
    w1T = W1[:, :, 1:].transpose(0, 2, 1)          # [p, c, m]
    w1s = np.zeros((4, 4, 128, 128), f)
    for g in range(4):
        w1s[:, g, 32 * g:32 * g + 32, :] = w1T
    w1s = w1s.astype(DT_np)
    w2s = np.ascontiguousarray(
        W2.transpose(0, 3, 4, 2, 1).reshape(4, 9, 128, 128)).astype(DT_np)
    w3s = np.ascontiguousarray(W3.transpose(0, 2, 1)).astype(DT_np)
    # augw[g, 32g+i, m] = aug_W[m, i] (i<3); zero elsewhere
    augw = np.zeros((4, 128, 32), f)
    for g in range(4):
        augw[g, 32 * g:32 * g + 3, :] = aug_W.T
    augw = augw.astype(DT_np)

    b1e = np.empty((128, 16), f)
    for eidx, (t, piece) in enumerate(_EVAL_TP):
        b1e[:, eidx] = b1[piece] + np.float32(t) * W1[piece][:, 0]
    b2s = np.ascontiguousarray(b2.T)                       # [128, 4]
    b3s = np.ascontiguousarray(np.tile(b3, (1, 4)).T)      # [128, 4]
    augb = np.tile(aug_b, 4)[:, None].astype(f)            # [128, 1]

    # row: [128 = t*32+c, q*10+cls] = ro_W[cls, c*1024 + t*256 + q]
    ro4 = ro_W.reshape(10, 32, 4, 256)                     # [cls, c, t, q]
    row = np.ascontiguousarray(
        ro4.transpose(2, 1, 3, 0).reshape(128, 2560)).astype(DT_np)
    rob = np.tile(ro_b, (16, 1)).astype(f)                 # [16, 10]

    # strip-sum selector: sel[32j + c, c] = 1 for c < 10
    sel = np.zeros((128, 10), f)
    for j in range(4):
        sel[32 * j:32 * j + 10, :] = np.eye(10, dtype=f)

    eye = np.eye(10, dtype=f)
    in_maps = []
    for k in range(NCORES):
        oneh = eye[y[k * BL:(k + 1) * BL]]
        in_maps.append({
            "xf": xf[k], "w1s": w1s, "w2s": w2s, "w3s": w3s, "augw": augw,
            "b1e": b1e, "b2s": b2s, "b3s": b3s, "augb": augb,
            "row": row, "sel": sel, "oneh": oneh, "rob": rob,
        })
    return in_maps


_NC_CACHE = {}


def _get_nc(debug=False):
    key = (DT_NAME, debug)
    if key not in _NC_CACHE:
        _NC_CACHE[key] = build_nc(debug)
    return _NC_CACHE[key]


def run(inputs, debug=False, **spmd_kwargs):
    nc = _get_nc(debug)
    in_maps = prep_in_maps(inputs)
    res = run_bass_kernel_spmd(nc, in_maps, core_ids=list(range(NCORES)),
                               **spmd_kwargs)
    loss = sum(r["outv"][0, 0] for r in res.results) / 128.0
    accu = sum(r["outv"][1, 0] for r in res.results)
    out = (np.asarray(loss, np.float32), np.asarray(accu, np.float32))
    return out, res


def kernel(**inputs):
    out, _ = run(inputs)
    return out
